# revision 13
# baseline (speedup 1.0000x reference)
"""Multi-head attention forward (b=8, n=2048, dim=512, heads=8, dh=64) on 8
Trainium2 NeuronCores.

Sharding: data-parallel over batch — core i computes the full attention layer
for batch element i (weights replicated, no collectives).

Per-core pipeline (transposed so softmax rowsums come out of the attnv
matmul via an appended ones-column on v):
  1. x [2048,512] -> cast bf16 -> PE-transpose -> xT [512,2048]
  2. qkT = w_qk.T @ xT           [1024,2048]  (q/k features on partitions)
  3. v   = x @ w_v               [2048,512]   (tokens on partitions) + ones col
  4. heads processed in PAIRS (2p, 2p+1) x query-halves of 1024:
       per key tile j: two K=64 sim matmuls run CONCURRENTLY in the PE array
       via row-group packing (head A in rows 0-63, head B in rows 64-127),
       writing one fp32 PSUM tile [128, 2, 1024]; ONE 2048-wide exp on the
       scalar engine; attnv accumulates [65, 1024] per head (row 64 = rowsum).
  5. normalization: denominators from the 4 (head, half) blocks of a pair are
       gathered onto partitions 0/32/64/96 of one tile (32-aligned partition
       shifts), one reciprocal_approx_fast covers all four, rows are shifted
       back to partition 0, gpsimd broadcasts to 128 partitions, DVE multiply
       writes normalized bf16 attn_outT.
  6. out = attn_outT.T @ w_out   [2048,512]

All matmul operands are bf16 (full-rate 1 col/cycle streaming; f32r lowers to
fp32_mode=HIGH at 2 cycles/row). PSUM is fp32 (TRN2 has no 16-bit PSUM), so
the 8 banks hold one sim tile (4) + two head accumulators (4) and the next
key-tile's sim waits for the exp read — a ~430ns serial insert per 2us exp.
"""

import numpy as np

import concourse.bass as bass
import concourse.mybir as mybir
import concourse.tile as tile
from concourse import bacc
from concourse.masks import make_identity

FP32 = mybir.dt.float32
BF16 = mybir.dt.bfloat16

B = 8
N = 2048
D = 512
H = 8
DH = 64
F3 = 3 * D
SCALE = DH**-0.5
P = 128
NT = N // P  # 16 token tiles
CT = D // P  # 4 contraction tiles over dim


def _attention_body(tc: "tile.TileContext", repeat: int = 1):
    nc = tc.nc
    x = nc.dram_tensor("x", [N, D], FP32, kind="ExternalInput").ap()
    w_qkv = nc.dram_tensor("w_qkv", [D, F3], FP32, kind="ExternalInput").ap()
    w_out = nc.dram_tensor("w_out", [D, D], FP32, kind="ExternalInput").ap()
    out = nc.dram_tensor("out", [N, D], FP32, kind="ExternalOutput").ap()

    with (
        tc.tile_pool(name="const", bufs=1) as const,
        tc.tile_pool(name="persist", bufs=1) as persist,
        tc.tile_pool(name="wstage", bufs=2) as wstage,
    ):
        # loop-invariant across repeats: constants and the two weight
        # matrices are loaded/cast exactly once
        identity = const.tile([P, P], BF16)
        make_identity(nc, identity)
        ones32 = const.tile([P, 1], FP32)
        nc.vector.memset(ones32, 1.0)

        # q and k features transposed: rows = 1024 q/k features in 8 tiles
        qkT = persist.tile([P, 8, N], BF16)
        # v with tokens on partitions; per head 64 value cols + 1 ones col
        v_aug = persist.tile([P, NT, H * 65], BF16)
        nc.vector.tensor_copy(
            out=v_aug.rearrange("p j (h c) -> p j h c", c=65)[:, :, :, 64:65],
            in_=ones32.to_broadcast([P, NT, H, 1]),
        )
        # normalized attention output, heads stacked in pairs:
        # tile t holds heads (2t, 2t+1) at rows 0-63 / 64-127
        attn_outT = persist.tile([P, CT, N], BF16)
        wout_sb = persist.tile([P, CT, D], BF16)
        for t in range(CT):
            ws = wstage.tile([P, F3], FP32, tag="ws")
            nc.sync.dma_start(out=ws[:, :D], in_=w_out[t * P : (t + 1) * P, :])
            nc.vector.tensor_copy(out=wout_sb[:, t, :], in_=ws[:, :D])
        wqkv_sb = persist.tile([P, CT, F3], BF16)
        for t in range(CT):
            ws = wstage.tile([P, F3], FP32, tag="ws")
            nc.sync.dma_start(out=ws, in_=w_qkv[t * P : (t + 1) * P, :])
            nc.vector.tensor_copy(out=wqkv_sb[:, t, :], in_=ws)

        consts = (identity, ones32, qkT, v_aug, attn_outT, wout_sb, wqkv_sb)
        for _ in range(repeat):
            _attention_once(tc, x, out, consts)


def _attention_once(tc: "tile.TileContext", x, out, consts):
    nc = tc.nc
    exp_f = mybir.ActivationFunctionType.Exp
    identity, ones32, qkT, v_aug, attn_outT, wout_sb, wqkv_sb = consts

    if True:
        with (
            tc.tile_pool(name="proj", bufs=1) as proj_pool,
            tc.tile_pool(name="xstage", bufs=3) as xstage,
            tc.tile_pool(name="xbfst", bufs=3) as xbfst,
            tc.tile_pool(name="pst", bufs=2, space="PSUM") as pst,
            tc.tile_pool(name="psmm", bufs=3, space="PSUM") as psmm,
        ):
            # ---- load x, cast bf16, transpose to xT [512, 2048] ----
            xT = proj_pool.tile([P, CT, N], BF16)
            for j in range(NT):
                xs = xstage.tile([P, D], FP32)
                nc.sync.dma_start(out=xs, in_=x[j * P : (j + 1) * P, :])
                xbf = xbfst.tile([P, D], BF16)
                nc.vector.tensor_copy(out=xbf, in_=xs)
                ps = pst.tile([P, CT, P], BF16)
                for t in range(CT):
                    nc.tensor.transpose(
                        ps[:, t, :], xbf[:, t * P : (t + 1) * P], identity
                    )
                nc.vector.tensor_copy(out=xT[:, :, j * P : (j + 1) * P], in_=ps)

            # ---- qkT (pair-0 q/k tiles first) and v = x @ w_v ----
            # qk m=0/4 runs first so pair 0's attention starts ASAP; their
            # psum copies (and v jp=0) ride the still-idle scalar engine
            def qk_tile(m, nbp):
                ps = psmm.tile([P, 2, 512], FP32, tag="mm")
                for c in range(CT):
                    for q in range(2):
                        nc.tensor.matmul(
                            ps[:, q, :],
                            wqkv_sb[:, c, m * P : (m + 1) * P],
                            xT[:, c, (nbp * 2 + q) * 512 : (nbp * 2 + q + 1) * 512],
                            start=(c == 0),
                            stop=(c == CT - 1),
                        )
                nc.vector.tensor_copy(
                    out=qkT[:, m, nbp * 1024 : (nbp + 1) * 1024],
                    in_=ps.rearrange("p a b -> p (a b)"),
                )

            def v_tile(jp):
                ps = psmm.tile([P, 2, 512], FP32, tag="mm")
                for c in range(CT):
                    for q in range(2):
                        j = jp * 2 + q
                        nc.tensor.matmul(
                            ps[:, q, :],
                            xT[:, c, j * P : (j + 1) * P],
                            wqkv_sb[:, c, 2 * D : 3 * D],
                            start=(c == 0),
                            stop=(c == CT - 1),
                        )
                dst = v_aug[:, jp * 2 : jp * 2 + 2, :].rearrange(
                    "p j (h c) -> p j h c", c=65
                )[:, :, :, 0:64]
                srcv = ps.rearrange("p q (h c) -> p q h c", c=64)
                nc.vector.tensor_copy(out=dst, in_=srcv)

            for jp in range(8):
                v_tile(jp)
            for m in (0, 4, 1, 5, 2, 6, 3, 7):
                for nbp in range(2):
                    qk_tile(m, nbp)

        # ---- attention: head pairs, query halves of 1024 ----
        with (
            tc.tile_pool(name="sim", bufs=1, space="PSUM") as simp,
            tc.tile_pool(name="psoA", bufs=1, space="PSUM") as psoAp,
            tc.tile_pool(name="psoB", bufs=1, space="PSUM") as psoBp,
            tc.tile_pool(name="exp", bufs=3) as expp,
            tc.tile_pool(name="unn", bufs=2) as unnp,
            tc.tile_pool(name="den", bufs=2) as denp,
            tc.tile_pool(name="rinv", bufs=2) as rinvp,
            tc.tile_pool(name="r0", bufs=3) as r0p,
            tc.tile_pool(name="db", bufs=3) as dbp,
        ):
            for pr in range(H // 2):
                hA, hB = 2 * pr, 2 * pr + 1
                qt, kt = pr, 4 + pr
                den_t = denp.tile([97, 1024], FP32)
                unn_pr = unnp.tile([P, N], BF16)
                for p2 in range(2):
                    psoA = psoAp.tile([65, 2, 512], FP32)
                    psoB = psoBp.tile([65, 2, 512], FP32)

                    def attnv(ex_t, pj):
                        for q in range(2):
                            nc.tensor.matmul(
                                psoA[:, q, :],
                                v_aug[:, pj, hA * 65 : (hA + 1) * 65],
                                ex_t[:, 0, q * 512 : (q + 1) * 512],
                                start=(pj == 0),
                                stop=(pj == NT - 1),
                            )
                        for q in range(2):
                            nc.tensor.matmul(
                                psoB[:, q, :],
                                v_aug[:, pj, hB * 65 : (hB + 1) * 65],
                                ex_t[:, 1, q * 512 : (q + 1) * 512],
                                start=(pj == 0),
                                stop=(pj == NT - 1),
                            )

                    pending = None
                    for j in range(NT):
                        simps = simp.tile([P, 2, 1024], FP32, tag="sim")
                        for q in range(2):
                            qsl = (p2 * 2 + q) * 512
                            nc.tensor.matmul(
                                simps[:, 0, q * 512 : (q + 1) * 512],
                                qkT[0:64, kt, j * P : (j + 1) * P],
                                qkT[0:64, qt, qsl : qsl + 512],
                                start=True,
                                stop=True,
                                tile_position=(0, 0),
                            )
                            nc.tensor.matmul(
                                simps[:, 1, q * 512 : (q + 1) * 512],
                                qkT[64:128, kt, j * P : (j + 1) * P],
                                qkT[64:128, qt, qsl : qsl + 512],
                                start=True,
                                stop=True,
                                tile_position=(64, 0),
                            )
                        ex_t = expp.tile([P, 2, 1024], BF16, tag="ex")
                        nc.scalar.activation(
                            out=ex_t.rearrange("p a b -> p (a b)"),
                            in_=simps.rearrange("p a b -> p (a b)"),
                            func=exp_f,
                            scale=SCALE,
                        )
                        if pending is not None:
                            attnv(*pending)
                        pending = (ex_t, j)
                    attnv(*pending)

                    # denominators to partitions {0,32} + p2*64; unnormalized
                    # out to SBUF (frees the PSUM accumulators)
                    nc.vector.tensor_copy(
                        out=den_t[64 * p2 : 64 * p2 + 1, :],
                        in_=psoA[64:65, :, :].rearrange("p a b -> p (a b)"),
                    )
                    nc.vector.tensor_copy(
                        out=den_t[64 * p2 + 32 : 64 * p2 + 33, :],
                        in_=psoB[64:65, :, :].rearrange("p a b -> p (a b)"),
                    )
                    nc.vector.tensor_copy(
                        out=unn_pr[0:64, p2 * 1024 : (p2 + 1) * 1024],
                        in_=psoA[0:64, :, :].rearrange("p a b -> p (a b)"),
                    )
                    nc.vector.tensor_copy(
                        out=unn_pr[64:128, p2 * 1024 : (p2 + 1) * 1024],
                        in_=psoB[0:64, :, :].rearrange("p a b -> p (a b)"),
                    )

                # one reciprocal covers the pair's 4 (head, half) rows
                rinv_t = rinvp.tile([97, 1024], FP32)
                nc.vector.reciprocal_approx_fast(out=rinv_t, in_=den_t)
                for row, qo, p2o in ((0, 0, 0), (32, 64, 0), (64, 0, 1), (96, 64, 1)):
                    if row == 0:
                        r0_ap = rinv_t[0:1, :]
                    else:
                        r0_t = r0p.tile([1, 1024], FP32, tag="r0")
                        nc.vector.tensor_copy(out=r0_t, in_=rinv_t[row : row + 1, :])
                        r0_ap = r0_t
                    db_t = dbp.tile([P, 1024], FP32, tag="db")
                    nc.gpsimd.partition_broadcast(db_t, r0_ap)
                    with nc.allow_low_precision("bf16 attn out"):
                        nc.vector.tensor_mul(
                            out=attn_outT[qo : qo + 64, pr, p2o * 1024 : (p2o + 1) * 1024],
                            in0=unn_pr[qo : qo + 64, p2o * 1024 : (p2o + 1) * 1024],
                            in1=db_t[qo : qo + 64, :],
                        )

        # ---- out = attn_outT.T @ w_out ----
        with (
            tc.tile_pool(name="pso2", bufs=2, space="PSUM") as pso2p,
            tc.tile_pool(name="outstage", bufs=2) as outstage,
        ):
            for jp in range(8):
                ps = pso2p.tile([P, 2, 512], FP32, tag="mm")
                for t in range(CT):
                    for q in range(2):
                        j = jp * 2 + q
                        nc.tensor.matmul(
                            ps[:, q, :],
                            attn_outT[:, t, j * P : (j + 1) * P],
                            wout_sb[:, t, :],
                            start=(t == 0),
                            stop=(t == CT - 1),
                        )
                os_ = outstage.tile([P, 2, D], FP32)
                nc.vector.tensor_copy(out=os_, in_=ps)
                nc.sync.dma_start(
                    out=out[jp * 256 : (jp + 1) * 256, :].rearrange(
                        "(q p) d -> p q d", p=P
                    ),
                    in_=os_,
                )


_CACHE: dict = {}


def build_nc(repeat: int = 1) -> "bass.Bass":
    key = ("nc", repeat)
    if key not in _CACHE:
        nc = bacc.Bacc("TRN2", target_bir_lowering=False, debug=False)
        with tile.TileContext(nc) as tc:
            _attention_body(tc, repeat=repeat)
        nc.compile()
        _CACHE[key] = nc
    return _CACHE[key]


def kernel(x: np.ndarray, w_qkv: np.ndarray, w_out: np.ndarray) -> np.ndarray:
    from concourse.bass_utils import run_bass_kernel_spmd

    nc = build_nc()
    x = np.ascontiguousarray(np.asarray(x, dtype=np.float32))
    w_qkv = np.ascontiguousarray(np.asarray(w_qkv, dtype=np.float32))
    w_out = np.ascontiguousarray(np.asarray(w_out, dtype=np.float32))
    in_maps = [
        {"x": x[i], "w_qkv": w_qkv, "w_out": w_out} for i in range(B)
    ]
    res = run_bass_kernel_spmd(nc, in_maps, core_ids=list(range(B)))
    return np.stack([r["out"] for r in res.results], axis=0)


# revision 14
# speedup vs baseline: 1.1945x; 1.1945x over previous
"""Multi-head attention forward (b=8, n=2048, dim=512, heads=8, dh=64) on 8
Trainium2 NeuronCores.

Sharding: data-parallel over batch — core i computes the full attention layer
for batch element i (weights replicated, no collectives).

Per-core pipeline (transposed so softmax rowsums come out of the attnv
matmul via an appended ones-column on v):
  1. x [2048,512] -> cast bf16 -> PE-transpose -> xT [512,2048]
  2. qkT = w_qk.T @ xT           [1024,2048]  (q/k features on partitions)
  3. v   = x @ w_v               [2048,512]   (tokens on partitions) + ones col
  4. heads processed in PAIRS (2p, 2p+1) x query-halves of 1024:
       per key tile j: two K=64 sim matmuls run CONCURRENTLY in the PE array
       via row-group packing (head A in rows 0-63, head B in rows 64-127),
       writing one fp32 PSUM tile [128, 2, 1024]; ONE 2048-wide exp on the
       scalar engine; attnv accumulates [65, 1024] per head (row 64 = rowsum).
  5. normalization: denominators from the 4 (head, half) blocks of a pair are
       gathered onto partitions 0/32/64/96 of one tile (32-aligned partition
       shifts), one reciprocal_approx_fast covers all four, rows are shifted
       back to partition 0, gpsimd broadcasts to 128 partitions, DVE multiply
       writes normalized bf16 attn_outT.
  6. out = attn_outT.T @ w_out   [2048,512]

All matmul operands are bf16 (full-rate 1 col/cycle streaming; f32r lowers to
fp32_mode=HIGH at 2 cycles/row). PSUM is fp32 (TRN2 has no 16-bit PSUM), so
the 8 banks hold one sim tile (4) + two head accumulators (4) and the next
key-tile's sim waits for the exp read — a ~430ns serial insert per 2us exp.
"""

import numpy as np

import concourse.bass as bass
import concourse.mybir as mybir
import concourse.tile as tile
from concourse import bacc
from concourse.masks import make_identity

FP32 = mybir.dt.float32
BF16 = mybir.dt.bfloat16

B = 8
N = 2048
D = 512
H = 8
DH = 64
F3 = 3 * D
SCALE = DH**-0.5
P = 128
NT = N // P  # 16 token tiles
CT = D // P  # 4 contraction tiles over dim


def _attention_body(tc: "tile.TileContext", repeat: int = 1):
    nc = tc.nc
    x = nc.dram_tensor("x", [N, D], FP32, kind="ExternalInput").ap()
    w_qkv = nc.dram_tensor("w_qkv", [D, F3], FP32, kind="ExternalInput").ap()
    w_out = nc.dram_tensor("w_out", [D, D], FP32, kind="ExternalInput").ap()
    out = nc.dram_tensor("out", [N, D], FP32, kind="ExternalOutput").ap()

    with (
        tc.tile_pool(name="const", bufs=1) as const,
        tc.tile_pool(name="persist", bufs=1) as persist,
        tc.tile_pool(name="wstage", bufs=2) as wstage,
    ):
        # loop-invariant across repeats: constants and the two weight
        # matrices are loaded/cast exactly once
        identity = const.tile([P, P], BF16)
        make_identity(nc, identity)
        ones32 = const.tile([P, 1], FP32)
        nc.vector.memset(ones32, 1.0)

        # q and k features transposed: rows = 1024 q/k features in 8 tiles
        qkT = persist.tile([P, 8, N], BF16)
        # v with tokens on partitions; per head 64 value cols + 1 ones col
        v_aug = persist.tile([P, NT, H * 65], BF16)
        nc.vector.tensor_copy(
            out=v_aug.rearrange("p j (h c) -> p j h c", c=65)[:, :, :, 64:65],
            in_=ones32.to_broadcast([P, NT, H, 1]),
        )
        # normalized attention output, heads stacked in pairs:
        # tile t holds heads (2t, 2t+1) at rows 0-63 / 64-127
        attn_outT = persist.tile([P, CT, N], BF16)
        wout_sb = persist.tile([P, CT, D], BF16)
        for t in range(CT):
            ws = wstage.tile([P, F3], FP32, tag="ws")
            nc.sync.dma_start(out=ws[:, :D], in_=w_out[t * P : (t + 1) * P, :])
            nc.vector.tensor_copy(out=wout_sb[:, t, :], in_=ws[:, :D])
        wqkv_sb = persist.tile([P, CT, F3], BF16)
        for t in range(CT):
            ws = wstage.tile([P, F3], FP32, tag="ws")
            nc.sync.dma_start(out=ws, in_=w_qkv[t * P : (t + 1) * P, :])
            nc.vector.tensor_copy(out=wqkv_sb[:, t, :], in_=ws)

        consts = (identity, ones32, qkT, v_aug, attn_outT, wout_sb, wqkv_sb)
        for _ in range(repeat):
            _attention_once(tc, x, out, consts)


def _attention_once(tc: "tile.TileContext", x, out, consts):
    nc = tc.nc
    exp_f = mybir.ActivationFunctionType.Exp
    identity, ones32, qkT, v_aug, attn_outT, wout_sb, wqkv_sb = consts

    if True:
        with (
            tc.tile_pool(name="proj", bufs=1) as proj_pool,
            tc.tile_pool(name="xstage", bufs=16) as xstage,
            tc.tile_pool(name="xbfst", bufs=6) as xbfst,
            tc.tile_pool(name="pst", bufs=2, space="PSUM") as pst,
            tc.tile_pool(name="psmm", bufs=3, space="PSUM") as psmm,
        ):
            # ---- load x, cast bf16, transpose to xT [512, 2048] ----
            xT = proj_pool.tile([P, CT, N], BF16)
            for j in range(NT):
                xs = xstage.tile([P, D], FP32)
                nc.sync.dma_start(out=xs, in_=x[j * P : (j + 1) * P, :])
                xbf = xbfst.tile([P, D], BF16)
                nc.vector.tensor_copy(out=xbf, in_=xs)
                ps = pst.tile([P, CT, P], BF16)
                for t in range(CT):
                    nc.tensor.transpose(
                        ps[:, t, :], xbf[:, t * P : (t + 1) * P], identity
                    )
                nc.vector.tensor_copy(out=xT[:, :, j * P : (j + 1) * P], in_=ps)

            # ---- qkT (pair-0 q/k tiles first) and v = x @ w_v ----
            # qk m=0/4 runs first so pair 0's attention starts ASAP; their
            # psum copies (and v jp=0) ride the still-idle scalar engine
            def qk_tile(m, nbp):
                ps = psmm.tile([P, 2, 512], FP32, tag="mm")
                for c in range(CT):
                    for q in range(2):
                        nc.tensor.matmul(
                            ps[:, q, :],
                            wqkv_sb[:, c, m * P : (m + 1) * P],
                            xT[:, c, (nbp * 2 + q) * 512 : (nbp * 2 + q + 1) * 512],
                            start=(c == 0),
                            stop=(c == CT - 1),
                        )
                nc.vector.tensor_copy(
                    out=qkT[:, m, nbp * 1024 : (nbp + 1) * 1024],
                    in_=ps.rearrange("p a b -> p (a b)"),
                )

            def v_tile(jp):
                ps = psmm.tile([P, 2, 512], FP32, tag="mm")
                for c in range(CT):
                    for q in range(2):
                        j = jp * 2 + q
                        nc.tensor.matmul(
                            ps[:, q, :],
                            xT[:, c, j * P : (j + 1) * P],
                            wqkv_sb[:, c, 2 * D : 3 * D],
                            start=(c == 0),
                            stop=(c == CT - 1),
                        )
                dst = v_aug[:, jp * 2 : jp * 2 + 2, :].rearrange(
                    "p j (h c) -> p j h c", c=65
                )[:, :, :, 0:64]
                srcv = ps.rearrange("p q (h c) -> p q h c", c=64)
                nc.vector.tensor_copy(out=dst, in_=srcv)

            for jp in range(8):
                v_tile(jp)
            for m in (0, 4, 1, 5, 2, 6, 3, 7):
                for nbp in range(2):
                    qk_tile(m, nbp)

        # ---- attention: head pairs, query halves of 1024 ----
        with (
            tc.tile_pool(name="sim", bufs=1, space="PSUM") as simp,
            tc.tile_pool(name="psoA", bufs=1, space="PSUM") as psoAp,
            tc.tile_pool(name="psoB", bufs=1, space="PSUM") as psoBp,
            tc.tile_pool(name="exp", bufs=3) as expp,
            tc.tile_pool(name="unn", bufs=2) as unnp,
            tc.tile_pool(name="den", bufs=2) as denp,
            tc.tile_pool(name="rinv", bufs=2) as rinvp,
            tc.tile_pool(name="r0", bufs=3) as r0p,
            tc.tile_pool(name="db", bufs=3) as dbp,
        ):
            for pr in range(H // 2):
                hA, hB = 2 * pr, 2 * pr + 1
                qt, kt = pr, 4 + pr
                den_t = denp.tile([97, 1024], FP32)
                unn_pr = unnp.tile([P, N], BF16)
                for p2 in range(2):
                    psoA = psoAp.tile([65, 2, 512], FP32)
                    psoB = psoBp.tile([65, 2, 512], FP32)

                    def attnv(ex_t, pj):
                        for q in range(2):
                            nc.tensor.matmul(
                                psoA[:, q, :],
                                v_aug[:, pj, hA * 65 : (hA + 1) * 65],
                                ex_t[:, 0, q * 512 : (q + 1) * 512],
                                start=(pj == 0),
                                stop=(pj == NT - 1),
                            )
                        for q in range(2):
                            nc.tensor.matmul(
                                psoB[:, q, :],
                                v_aug[:, pj, hB * 65 : (hB + 1) * 65],
                                ex_t[:, 1, q * 512 : (q + 1) * 512],
                                start=(pj == 0),
                                stop=(pj == NT - 1),
                            )

                    pending = None
                    for j in range(NT):
                        simps = simp.tile([P, 2, 1024], FP32, tag="sim")
                        for q in range(2):
                            qsl = (p2 * 2 + q) * 512
                            nc.tensor.matmul(
                                simps[:, 0, q * 512 : (q + 1) * 512],
                                qkT[0:64, kt, j * P : (j + 1) * P],
                                qkT[0:64, qt, qsl : qsl + 512],
                                start=True,
                                stop=True,
                                tile_position=(0, 0),
                            )
                            nc.tensor.matmul(
                                simps[:, 1, q * 512 : (q + 1) * 512],
                                qkT[64:128, kt, j * P : (j + 1) * P],
                                qkT[64:128, qt, qsl : qsl + 512],
                                start=True,
                                stop=True,
                                tile_position=(64, 0),
                            )
                        ex_t = expp.tile([P, 2, 1024], BF16, tag="ex")
                        nc.scalar.activation(
                            out=ex_t.rearrange("p a b -> p (a b)"),
                            in_=simps.rearrange("p a b -> p (a b)"),
                            func=exp_f,
                            scale=SCALE,
                        )
                        if pending is not None:
                            attnv(*pending)
                        pending = (ex_t, j)
                    attnv(*pending)

                    # denominators to partitions {0,32} + p2*64; unnormalized
                    # out to SBUF (frees the PSUM accumulators)
                    nc.vector.tensor_copy(
                        out=den_t[64 * p2 : 64 * p2 + 1, :],
                        in_=psoA[64:65, :, :].rearrange("p a b -> p (a b)"),
                    )
                    nc.vector.tensor_copy(
                        out=den_t[64 * p2 + 32 : 64 * p2 + 33, :],
                        in_=psoB[64:65, :, :].rearrange("p a b -> p (a b)"),
                    )
                    nc.vector.tensor_copy(
                        out=unn_pr[0:64, p2 * 1024 : (p2 + 1) * 1024],
                        in_=psoA[0:64, :, :].rearrange("p a b -> p (a b)"),
                    )
                    nc.vector.tensor_copy(
                        out=unn_pr[64:128, p2 * 1024 : (p2 + 1) * 1024],
                        in_=psoB[0:64, :, :].rearrange("p a b -> p (a b)"),
                    )

                # one reciprocal covers the pair's 4 (head, half) rows
                rinv_t = rinvp.tile([97, 1024], FP32)
                nc.vector.reciprocal_approx_fast(out=rinv_t, in_=den_t)
                for row, qo, p2o in ((0, 0, 0), (32, 64, 0), (64, 0, 1), (96, 64, 1)):
                    if row == 0:
                        r0_ap = rinv_t[0:1, :]
                    else:
                        r0_t = r0p.tile([1, 1024], FP32, tag="r0")
                        nc.vector.tensor_copy(out=r0_t, in_=rinv_t[row : row + 1, :])
                        r0_ap = r0_t
                    db_t = dbp.tile([P, 1024], FP32, tag="db")
                    nc.gpsimd.partition_broadcast(db_t, r0_ap)
                    with nc.allow_low_precision("bf16 attn out"):
                        nc.vector.tensor_mul(
                            out=attn_outT[qo : qo + 64, pr, p2o * 1024 : (p2o + 1) * 1024],
                            in0=unn_pr[qo : qo + 64, p2o * 1024 : (p2o + 1) * 1024],
                            in1=db_t[qo : qo + 64, :],
                        )

        # ---- out = attn_outT.T @ w_out ----
        with (
            tc.tile_pool(name="pso2", bufs=2, space="PSUM") as pso2p,
            tc.tile_pool(name="outstage", bufs=2) as outstage,
        ):
            for jp in range(8):
                ps = pso2p.tile([P, 2, 512], FP32, tag="mm")
                for t in range(CT):
                    for q in range(2):
                        j = jp * 2 + q
                        nc.tensor.matmul(
                            ps[:, q, :],
                            attn_outT[:, t, j * P : (j + 1) * P],
                            wout_sb[:, t, :],
                            start=(t == 0),
                            stop=(t == CT - 1),
                        )
                os_ = outstage.tile([P, 2, D], FP32)
                nc.vector.tensor_copy(out=os_, in_=ps)
                nc.sync.dma_start(
                    out=out[jp * 256 : (jp + 1) * 256, :].rearrange(
                        "(q p) d -> p q d", p=P
                    ),
                    in_=os_,
                )


_CACHE: dict = {}


def build_nc(repeat: int = 1) -> "bass.Bass":
    key = ("nc", repeat)
    if key not in _CACHE:
        nc = bacc.Bacc("TRN2", target_bir_lowering=False, debug=False)
        with tile.TileContext(nc) as tc:
            _attention_body(tc, repeat=repeat)
        nc.compile()
        _CACHE[key] = nc
    return _CACHE[key]


def kernel(x: np.ndarray, w_qkv: np.ndarray, w_out: np.ndarray) -> np.ndarray:
    from concourse.bass_utils import run_bass_kernel_spmd

    nc = build_nc()
    x = np.ascontiguousarray(np.asarray(x, dtype=np.float32))
    w_qkv = np.ascontiguousarray(np.asarray(w_qkv, dtype=np.float32))
    w_out = np.ascontiguousarray(np.asarray(w_out, dtype=np.float32))
    in_maps = [
        {"x": x[i], "w_qkv": w_qkv, "w_out": w_out} for i in range(B)
    ]
    res = run_bass_kernel_spmd(nc, in_maps, core_ids=list(range(B)))
    return np.stack([r["out"] for r in res.results], axis=0)
